# revision 12
# baseline (speedup 1.0000x reference)
"""Tensor-parallel attention kernel for 8 trn2 NeuronCores (v3, fp16).

Strategy (tensor-parallel over heads):
  - each core owns 2 of the 16 heads: wq/wk/wv sharded column-wise,
    QKV projection + rope + attention fused per batch, Q/K/V kept in
    SBUF, fp16 matmul operands with fp32 PSUM accumulation,
  - per-core attention outputs (256 rows of attn^T) are AllGathered,
  - each core then computes a 256-column slice of the final output
    (wo sharded column-wise), host concatenates + transposes.

v3: one x DMA per token tile (host-interleaved layout), paired-kt
softmax (one exp per two score tiles), denominator via DVE f16
accumulation + one one-hot matmul per (qt,h), batched per-batch
reciprocal, loads on sync / stores on gpsimd queues, wo preloaded.
"""

import numpy as np

import concourse.bass as bass
import concourse.mybir as mybir
import concourse.tile as tile
from concourse import bacc
from concourse.bass_utils import run_bass_kernel_spmd

B, S, D, H = 4, 2048, 2048, 16
HD = D // H            # 128
N_CORES = 8
HPC = H // N_CORES     # heads per core = 2
T = B * S              # 8192 tokens
CW = HPC * HD          # per-core feature width = 256

TOK = 512              # q/token tile (free dim of matmuls)
KTILE = 128            # k-token tile (partition dim)
NTT = T // TOK         # 16 token tiles over all batches
NTB = S // TOK         # 4 token tiles per batch
NQT = S // TOK         # 4 q tiles per sequence
NKT = S // KTILE       # 16 k tiles per sequence
NKC = D // 128         # 16 contraction chunks for projections
XW = NKC * TOK         # x columns per token tile in interleaved layout

F32 = mybir.dt.float32
F32R = mybir.dt.float32r
F16 = mybir.dt.float16

_KERNEL_CACHE = {}


def _analyze_mask(mask):
    """Per (k_tile, q_tile): skip entirely-masked tiles, flag tiles needing
    the multiplicative mask. Works for causal, all-zero, and arbitrary."""
    m = mask.reshape(S, S)  # [q, k]
    mt = m.reshape(NQT, TOK, NKT, KTILE)
    tmax = mt.max(axis=(1, 3))  # [qt, kt]
    tmin = mt.min(axis=(1, 3))
    skip = tmax <= -1e8                      # exp underflows to exactly 0
    need = (~skip) & ((tmin != 0) | (tmax != 0))
    return skip.T, need.T                    # [kt, qt]


def _build(skip, need, n_mask_tiles, mask_uid):
    """Build the per-core Bass program. skip/need: [NKT, NQT] bool;
    mask_uid maps (kt, qt) -> index into the deduplicated mask-tile stack."""
    nc = bacc.Bacc("TRN2", target_bir_lowering=False, debug=False,
                   num_devices=N_CORES)

    # x interleaved on host: xT[p, tt*XW + kc*TOK + c] = x[tt*TOK+c, kc*128+p]
    xT = nc.declare_dram_parameter("xT", [128, NTT * XW], F16, isOutput=False)
    # weights pre-chunked on host to [128, NKC*CW] (chunk kc at cols kc*CW)
    wq = nc.declare_dram_parameter("wq", [128, NKC * CW], F16, isOutput=False)
    wk = nc.declare_dram_parameter("wk", [128, NKC * CW], F16, isOutput=False)
    wv = nc.declare_dram_parameter("wv", [128, NKC * CW], F16, isOutput=False)
    wo = nc.declare_dram_parameter("wo", [128, NKC * CW], F16, isOutput=False)
    ropeC = nc.declare_dram_parameter("ropeC", [128, S], F32, isOutput=False)
    ropeS = nc.declare_dram_parameter("ropeS", [128, S], F32, isOutput=False)
    perm = nc.declare_dram_parameter("perm", [128, 128], F16, isOutput=False)
    # oneh[:, 8r:8r+8] = one-hot col r (denominator row-select lhsT)
    oneh = nc.declare_dram_parameter("oneh", [128, 8 * (2 * NQT)], F16,
                                     isOutput=False)
    # sel[:, 128r:128(r+1)] = [8,128] with row r all-ones (bcast lhsT)
    sel = nc.declare_dram_parameter("sel", [8, 128 * (2 * NQT)], F32R,
                                    isOutput=False)
    if n_mask_tiles:
        maskt = nc.declare_dram_parameter(
            "maskt", [n_mask_tiles, KTILE, TOK], F16, isOutput=False)
    outT = nc.declare_dram_parameter("outT", [CW, T], F32, isOutput=True)

    # attention for q-tile qt can start once projections cover its last
    # active k-tile (for causal masks that is token-tile qt itself)
    maxtt = {}
    for qt in range(NQT):
        kts = [kt for kt in range(NKT) if not skip[kt, qt]]
        assert kts, f"fully masked q tile {qt}"
        maxtt[qt] = max(max(kts) // (TOK // KTILE), qt)
    attn_order = []
    for tl in range(NTB):
        for qt in range(NQT):
            if maxtt[qt] == tl:
                for h in range(HPC):
                    attn_order.append((qt, h))
    first_qh = attn_order[0]
    last_qh = attn_order[-1]

    inv_sqrt_hd = 1.0 / float(np.sqrt(HD))

    with tile.TileContext(nc) as tc:
        with tc.tile_pool(name="const", bufs=1) as const, \
             tc.tile_pool(name="dram", bufs=1, space="DRAM") as dram:
            # persistent SBUF constants; weights split into 4 DMAs so the
            # first projection matmuls start as soon as chunk 0 lands
            wq_sb = const.tile([128, NKC * CW], F16)
            wk_sb = const.tile([128, NKC * CW], F16)
            wv_sb = const.tile([128, NKC * CW], F16)
            wo_sb = const.tile([128, NKC * CW], F16)
            Wq = NKC * CW // 4
            # sync carries only wq + the x tiles so the first matmul
            # starts ~9us in; other consts ride scalar/gpsimd queues
            for sb, dr, eng in ((wq_sb, wq, nc.sync),
                                (wk_sb, wk, nc.scalar),
                                (wv_sb, wv, nc.scalar),
                                (wo_sb, wo, nc.gpsimd)):
                for j in range(4):
                    eng.dma_start(sb[:, j * Wq:(j + 1) * Wq],
                                  dr[:, j * Wq:(j + 1) * Wq])
            C_sb = const.tile([128, S], F32)
            S_sb = const.tile([128, S], F32)
            nc.gpsimd.dma_start(C_sb[:], ropeC[:])
            nc.gpsimd.dma_start(S_sb[:], ropeS[:])
            perm_sb = const.tile([128, 128], F16)
            oneh_sb = const.tile([128, 8 * (2 * NQT)], F16)
            sel_sb = const.tile([8, 128 * (2 * NQT)], F32R)
            nc.gpsimd.dma_start(perm_sb[:], perm[:])
            nc.gpsimd.dma_start(oneh_sb[:], oneh[:])
            nc.gpsimd.dma_start(sel_sb[:], sel[:])
            preload_mask = 0 < n_mask_tiles <= 8
            mask_sb = None
            if preload_mask:
                mask_sb = const.tile([128, n_mask_tiles * TOK], F16)
                for j in range(n_mask_tiles):
                    nc.scalar.dma_start(
                        mask_sb[:, j * TOK:(j + 1) * TOK], maskt[j])

            # internal DRAM: one AllGather buffer pair per batch so the
            # collectives overlap with later batches' compute
            ag_in = [dram.tile([CW, S], F16, name=f"agin{b}")
                     for b in range(B)]
            ag_out = [dram.tile([CW * N_CORES, S], F16, addr_space="Shared",
                                name=f"agout{b}")
                      for b in range(B)]

            # -------- fused per-batch: QKV projection + rope + attention ----
            with tc.tile_pool(name="fx", bufs=2) as fx, \
                 tc.tile_pool(name="fbig", bufs=2) as fbig, \
                 tc.tile_pool(name="facc", bufs=1, space="PSUM") as facc, \
                 tc.tile_pool(name="fscr", bufs=2, space="PSUM") as fscr, \
                 tc.tile_pool(name="fo", bufs=1, space="PSUM") as fo, \
                 tc.tile_pool(name="fd", bufs=1, space="PSUM") as fd, \
                 tc.tile_pool(name="fsb", bufs=2) as fsb, \
                 tc.tile_pool(name="fac", bufs=2) as fac, \
                 tc.tile_pool(name="fno", bufs=2 * NQT) as fno, \
                 tc.tile_pool(name="fex", bufs=3) as fex, \
                 tc.tile_pool(name="fxm", bufs=2) as fxm, \
                 tc.tile_pool(name="p3a", bufs=20) as p3a:

                def emit_norm(b, drc_b, o_raws_b):
                    # normalize + store batch b (deferred past the next
                    # batch's Q pass so the PE never waits on the recip)
                    for (qt, h) in attn_order:
                        r = 2 * qt + h
                        bc2 = fscr.tile([128, 2 * TOK], F32, tag="scr2",
                                        name="bc")
                        bc_ps = bc2[:, 0:TOK]
                        nc.tensor.matmul(
                            bc_ps, sel_sb[:, 128 * r:128 * (r + 1)],
                            drc_b[:], start=True, stop=True)
                        bcs = fsb.tile([128, TOK], F16, tag="bcs",
                                       name="bcs")
                        nc.vector.tensor_copy(bcs[:], bc_ps)
                        o_sb = fsb.tile([128, TOK], F16, tag="osb",
                                        name="osb")
                        nc.vector.tensor_mul(o_sb[:], o_raws_b[(qt, h)][:],
                                             bcs[:])
                        nc.gpsimd.dma_start(
                            ag_in[b][128 * h:128 * (h + 1),
                                     TOK * qt:TOK * (qt + 1)],
                            o_sb[:])
                    nc.gpsimd.collective_compute(
                        "AllGather", mybir.AluOpType.bypass,
                        ins=[ag_in[b].opt()], outs=[ag_out[b].opt()],
                        replica_groups=[list(range(N_CORES))],
                    )

                def emit_p3(tt):
                    # one output-projection token tile; single PSUM bank
                    # (tag "ops"), m-halves sequential over resident ach
                    pb, ptl = tt // NTB, tt % NTB
                    achs = []
                    for kc in range(NKC):
                        ach = p3a.tile([128, TOK], F16, tag="ach",
                                       name="ach")
                        nc_dma = nc.scalar if kc % 2 else nc.sync
                        nc_dma.dma_start(
                            ach[:],
                            ag_out[pb][128 * kc:128 * (kc + 1),
                                       TOK * ptl:TOK * (ptl + 1)])
                        achs.append(ach)
                    for m in range(HPC):
                        pss = fo.tile([128, TOK], F32, tag="ops",
                                      name="pss")
                        for kc in range(NKC):
                            c0 = kc * CW + m * 128
                            nc.tensor.matmul(pss[:],
                                             wo_sb[:, c0:c0 + 128],
                                             achs[kc][:],
                                             start=(kc == 0),
                                             stop=(kc == NKC - 1))
                        osb = fsb.tile([128, TOK], F32, tag="osb3",
                                       name="osb3")
                        nc.scalar.copy(osb[:], pss[:])
                        nc.gpsimd.dma_start(
                            outT[128 * m:128 * (m + 1),
                                 TOK * tt:TOK * (tt + 1)], osb[:])

                pending_norm = None     # (b, drc, o_raws) awaiting emission
                pending_p3 = []         # token tiles whose AG has triggered
                # how many p3 tiles to fold into each tl slot (skip tl=0,
                # where the deferred normalize runs)
                p3_quota = {0: 0, 1: 1, 2: 1, 3: 2}
                for b in range(B):
                    qT = [fbig.tile([128, S], F16, tag=f"qT{h}", name=f"qT{h}")
                          for h in range(HPC)]
                    kT = [fbig.tile([128, S], F16, tag=f"kT{h}", name=f"kT{h}")
                          for h in range(HPC)]
                    vnat = [fbig.tile([128, S], F16, tag=f"vn{h}",
                                      name=f"vn{h}")
                            for h in range(HPC)]
                    d_all = fd.tile([2 * NQT, TOK], F32, tag="dps",
                                    name="dall")
                    drc = fsb.tile([2 * NQT, TOK], F32R, tag="drc",
                                   name="drc")
                    o_raws = {}
                    for tl in range(NTB):
                        tt = NTB * b + tl
                        pos0 = tl * TOK
                        xbig = fx.tile([128, XW], F16, tag="xbig",
                                       name="xbig")
                        nc.sync.dma_start(
                            xbig[:], xT[:, XW * tt:XW * (tt + 1)])

                        def xsl(kc):
                            return xbig[:, TOK * kc:TOK * (kc + 1)]

                        # Q pass then K pass, 2 shared psum banks each;
                        # rope follows each pass
                        for nm, wsb, dsts in (("q", wq_sb, qT),
                                              ("k", wk_sb, kT)):
                            pp = [facc.tile([128, TOK], F32, tag=f"pa{hh}",
                                            name=f"p{nm}{hh}")
                                  for hh in range(HPC)]
                            for kc in range(NKC):
                                st = (kc == 0)
                                sp = (kc == NKC - 1)
                                for hh in range(HPC):
                                    c0 = kc * CW + hh * HD
                                    nc.tensor.matmul(pp[hh][:],
                                                     wsb[:, c0:c0 + HD],
                                                     xsl(kc),
                                                     start=st, stop=sp)
                            for hh in range(HPC):
                                raw = fsb.tile([128, TOK], F16, tag="raw",
                                               name="raw")
                                nc.vector.tensor_copy(raw[:], pp[hh][:])
                                sw2 = fscr.tile([128, 2 * TOK], F32,
                                                tag="scr2", name="swp")
                                swp = sw2[:, 0:TOK]
                                nc.tensor.matmul(swp, perm_sb[:], raw[:],
                                                 start=True, stop=True)
                                t1 = fsb.tile([128, TOK], F16, tag="t1",
                                              name="t1")
                                nc.vector.tensor_mul(
                                    t1[:], pp[hh][:], C_sb[:, pos0:pos0 + TOK])
                                t2 = fsb.tile([128, TOK], F16, tag="t2",
                                              name="t2")
                                nc.vector.tensor_mul(
                                    t2[:], swp, S_sb[:, pos0:pos0 + TOK])
                                nc.vector.tensor_add(
                                    dsts[hh][:, pos0:pos0 + TOK],
                                    t1[:], t2[:])
                        if tl == 0 and pending_norm is not None:
                            emit_norm(*pending_norm)
                            pending_norm = None
                            pending_p3.extend(
                                NTB * (b - 1) + i for i in range(NTB))
                        # V pass: project directly into [token, feat] layout
                        # (stationary = x sub-block, moving = wv chunk row)
                        for sub in range(TOK // KTILE):
                            kt = (pos0 // KTILE) + sub
                            pv = facc.tile([128, CW], F32, tag="pa0",
                                           name="pav")
                            for kc in range(NKC):
                                nc.tensor.matmul(
                                    pv[:],
                                    xsl(kc)[:, KTILE * sub:KTILE * (sub + 1)],
                                    wv_sb[:, kc * CW:(kc + 1) * CW],
                                    start=(kc == 0), stop=(kc == NKC - 1))
                            for hh in range(HPC):
                                nc.vector.tensor_copy(
                                    vnat[hh][:, KTILE * kt:KTILE * (kt + 1)],
                                    pv[:, HD * hh:HD * (hh + 1)])
                        # attention for every q-tile whose K/V coverage is
                        # now complete; kt tiles processed in pairs with a
                        # single exp over [128, 2*TOK]
                        for qt in range(NQT):
                          if maxtt[qt] != tl:
                            continue
                          for h in range(HPC):
                            r = 2 * qt + h
                            kts = [kt for kt in range(NKT) if not skip[kt, qt]]
                            groups = [kts[i:i + 2]
                                      for i in range(0, len(kts), 2)]
                            o_ps = fo.tile([128, TOK], F32, tag="ops",
                                           name="ops")
                            acc = None
                            nmm = len(kts)
                            j = 0
                            for grp in groups:
                                ng = len(grp)
                                s2 = fscr.tile([128, 2 * TOK], F32,
                                               tag="scr2", name="sps")
                                for gi, kt in enumerate(grp):
                                    nc.tensor.matmul(
                                        s2[:, TOK * gi:TOK * (gi + 1)],
                                        kT[h][:, KTILE * kt:KTILE * (kt + 1)],
                                        qT[h][:, TOK * qt:TOK * (qt + 1)],
                                        start=True, stop=True)
                                e2 = fex.tile([128, 2 * TOK], F16, tag="ex",
                                              name="ex")
                                nc.scalar.activation(
                                    e2[:, 0:TOK * ng], s2[:, 0:TOK * ng],
                                    mybir.ActivationFunctionType.Exp,
                                    scale=inv_sqrt_hd)
                                for gi, kt in enumerate(grp):
                                    ex = e2[:, TOK * gi:TOK * (gi + 1)]
                                    if need[kt, qt]:
                                        mj = mask_uid[(kt, qt)]
                                        if preload_mask:
                                            msrc = mask_sb[:, mj * TOK:
                                                           (mj + 1) * TOK]
                                        else:
                                            mld = fsb.tile(
                                                [128, TOK], F16,
                                                tag="mld", name="mld")
                                            nc.sync.dma_start(mld[:],
                                                              maskt[mj])
                                            msrc = mld[:]
                                        exm = fxm.tile([128, TOK], F16,
                                                       tag="exm", name="exm")
                                        nc.vector.tensor_mul(exm[:], ex,
                                                             msrc)
                                        ex = exm[:]
                                    nc.tensor.matmul(
                                        o_ps[:],
                                        vnat[h][:, KTILE * kt:
                                                KTILE * (kt + 1)],
                                        ex, start=(j == 0),
                                        stop=(j == nmm - 1))
                                    # f16 running sum for the denominator
                                    nacc = fac.tile([128, TOK], F16,
                                                    tag="acb", name="acb")
                                    if acc is None:
                                        nc.vector.tensor_copy(nacc[:], ex)
                                    else:
                                        nc.vector.tensor_add(nacc[:],
                                                             acc[:], ex)
                                    acc = nacc
                                    j += 1
                            # one column-sum matmul per (qt,h) into row r
                            nc.tensor.matmul(
                                d_all[:], oneh_sb[:, 8 * r:8 * r + 8],
                                acc[:],
                                start=((qt, h) == first_qh),
                                stop=((qt, h) == last_qh))
                            # unnormalized AV -> SBUF, freeing the PSUM bank
                            orw = fno.tile([128, TOK], F16, tag="oraw",
                                           name="oraw")
                            nc.vector.tensor_copy(orw[:], o_ps[:])
                            o_raws[(qt, h)] = orw
                            if (qt, h) == last_qh:
                                with nc.allow_low_precision(
                                        reason="fp32r softmax denom recip"):
                                    nc.vector.reciprocal(drc[:], d_all[:])
                        # fold deferred output-projection tiles into
                        # this tl slot (PE-dense filler work)
                        for _ in range(p3_quota[tl]):
                            if pending_p3:
                                emit_p3(pending_p3.pop(0))
                    pending_norm = (b, drc, o_raws)
                # drain: last batch's normalize + AG, then its p3 tiles
                emit_norm(*pending_norm)
                pending_norm = None
                pending_p3.extend(NTB * (B - 1) + i for i in range(NTB))
                while pending_p3:
                    emit_p3(pending_p3.pop(0))

    nc.compile()
    return nc


def prepare(x, wq, wk, wv, wo, freqs_cos, freqs_sin, mask, cache_k, cache_v,
            start_pos):
    """Compile (cached) and build per-core input maps."""
    assert int(start_pos) == 0, "kernel compiled for start_pos == 0"
    x = np.asarray(x, dtype=np.float32)
    wq = np.asarray(wq, dtype=np.float32)
    wk = np.asarray(wk, dtype=np.float32)
    wv = np.asarray(wv, dtype=np.float32)
    wo = np.asarray(wo, dtype=np.float32)
    fc = np.asarray(freqs_cos, dtype=np.float32)
    fs = np.asarray(freqs_sin, dtype=np.float32)
    mask = np.asarray(mask, dtype=np.float32)

    skip, need = _analyze_mask(mask)
    with np.errstate(under="ignore", over="ignore"):
        mT = np.exp(mask.reshape(S, S).T.astype(np.float64))  # [k, q] mult
    uniq, mask_uid, tiles = {}, {}, []
    for kt in range(NKT):
        for qt in range(NQT):
            if need[kt, qt]:
                tl = np.ascontiguousarray(
                    mT[KTILE * kt:KTILE * (kt + 1),
                       TOK * qt:TOK * (qt + 1)]).astype(np.float16)
                hkey = tl.tobytes()
                if hkey not in uniq:
                    uniq[hkey] = len(tiles)
                    tiles.append(tl)
                mask_uid[(kt, qt)] = uniq[hkey]
    n_mask_tiles = len(tiles)

    key = (skip.tobytes(), need.tobytes(),
           tuple(sorted(mask_uid.items())))
    if key not in _KERNEL_CACHE:
        _KERNEL_CACHE[key] = _build(skip, need, n_mask_tiles, mask_uid)
    nc = _KERNEL_CACHE[key]

    # host-side input marshalling: x -> [128, tt, kc, c] interleaved
    x16 = x.reshape(T, D).astype(np.float16)
    xT = np.ascontiguousarray(
        x16.reshape(NTT, TOK, NKC, 128).transpose(3, 0, 2, 1).reshape(
            128, NTT * XW))
    C = np.repeat(fc.T, 2, axis=0).astype(np.float32)                # [128, S]
    Ssg = np.repeat(fs.T, 2, axis=0).astype(np.float32)
    Ssg[0::2] *= -1.0
    pm = np.zeros((128, 128), np.float16)
    idx = np.arange(0, 128, 2)
    pm[idx, idx + 1] = 1.0
    pm[idx + 1, idx] = 1.0
    oneh = np.zeros((128, 8 * (2 * NQT)), np.float16)
    sel = np.zeros((8, 128 * (2 * NQT)), np.float32)
    for r in range(2 * NQT):
        oneh[:, 8 * r + r] = 1.0
        sel[r, 128 * r:128 * (r + 1)] = 1.0
    maskt = (np.ascontiguousarray(np.stack(tiles)) if tiles
             else np.zeros((0, KTILE, TOK), np.float16))

    def chunk_w(w):  # [D, CW] -> [128, NKC*CW], fp16
        return np.ascontiguousarray(
            w.reshape(NKC, 128, CW).transpose(1, 0, 2).reshape(
                128, NKC * CW).astype(np.float16))

    in_maps = []
    for i in range(N_CORES):
        cols = slice(CW * i, CW * (i + 1))
        m = {
            "xT": xT,
            "wq": chunk_w(wq[:, cols]),
            "wk": chunk_w(wk[:, cols]),
            "wv": chunk_w(wv[:, cols]),
            "wo": chunk_w(wo[:, cols]),
            "ropeC": C, "ropeS": Ssg,
            "perm": pm, "oneh": oneh, "sel": sel,
        }
        if n_mask_tiles:
            m["maskt"] = maskt
        in_maps.append(m)
    return nc, in_maps


def kernel(**inputs):
    nc, in_maps = prepare(**inputs)
    res = run_bass_kernel_spmd(nc, in_maps, list(range(N_CORES)))
    outT = np.concatenate([res.results[i]["outT"] for i in range(N_CORES)],
                          axis=0)  # [D, T]
    return np.ascontiguousarray(outT.T).reshape(B, S, D)


# revision 14
# speedup vs baseline: 1.1033x; 1.1033x over previous
"""Tensor-parallel attention kernel for 8 trn2 NeuronCores (v3, fp16).

Strategy (tensor-parallel over heads):
  - each core owns 2 of the 16 heads: wq/wk/wv sharded column-wise,
    QKV projection + rope + attention fused per batch, Q/K/V kept in
    SBUF, fp16 matmul operands with fp32 PSUM accumulation,
  - per-core attention outputs (256 rows of attn^T) are AllGathered,
  - each core then computes a 256-column slice of the final output
    (wo sharded column-wise), host concatenates + transposes.

v3: one x DMA per token tile (host-interleaved layout), paired-kt
softmax (one exp per two score tiles), denominator via DVE f16
accumulation + one one-hot matmul per (qt,h), batched per-batch
reciprocal, loads on sync / stores on gpsimd queues, wo preloaded.
"""

import numpy as np

import concourse.bass as bass
import concourse.mybir as mybir
import concourse.tile as tile
from concourse import bacc
from concourse.bass_utils import run_bass_kernel_spmd

B, S, D, H = 4, 2048, 2048, 16
HD = D // H            # 128
N_CORES = 8
HPC = H // N_CORES     # heads per core = 2
T = B * S              # 8192 tokens
CW = HPC * HD          # per-core feature width = 256

TOK = 512              # q/token tile (free dim of matmuls)
KTILE = 128            # k-token tile (partition dim)
NTT = T // TOK         # 16 token tiles over all batches
NTB = S // TOK         # 4 token tiles per batch
NQT = S // TOK         # 4 q tiles per sequence
NKT = S // KTILE       # 16 k tiles per sequence
NKC = D // 128         # 16 contraction chunks for projections
XW = NKC * TOK         # x columns per token tile in interleaved layout

F32 = mybir.dt.float32
F32R = mybir.dt.float32r
F16 = mybir.dt.float16

_KERNEL_CACHE = {}


def _analyze_mask(mask):
    """Per (k_tile, q_tile): skip entirely-masked tiles, flag tiles needing
    the multiplicative mask. Works for causal, all-zero, and arbitrary."""
    m = mask.reshape(S, S)  # [q, k]
    mt = m.reshape(NQT, TOK, NKT, KTILE)
    tmax = mt.max(axis=(1, 3))  # [qt, kt]
    tmin = mt.min(axis=(1, 3))
    skip = tmax <= -1e8                      # exp underflows to exactly 0
    need = (~skip) & ((tmin != 0) | (tmax != 0))
    return skip.T, need.T                    # [kt, qt]


def _build(skip, need, n_mask_tiles, mask_uid):
    """Build the per-core Bass program. skip/need: [NKT, NQT] bool;
    mask_uid maps (kt, qt) -> index into the deduplicated mask-tile stack."""
    nc = bacc.Bacc("TRN2", target_bir_lowering=False, debug=False,
                   num_devices=N_CORES)

    # x interleaved on host: xT[p, tt*XW + kc*TOK + c] = x[tt*TOK+c, kc*128+p]
    xT = nc.declare_dram_parameter("xT", [128, NTT * XW], F16, isOutput=False)
    # weights pre-chunked on host to [128, NKC*CW] (chunk kc at cols kc*CW)
    wq = nc.declare_dram_parameter("wq", [128, NKC * CW], F16, isOutput=False)
    wk = nc.declare_dram_parameter("wk", [128, NKC * CW], F16, isOutput=False)
    wv = nc.declare_dram_parameter("wv", [128, NKC * CW], F16, isOutput=False)
    wo = nc.declare_dram_parameter("wo", [128, NKC * CW], F16, isOutput=False)
    ropeC = nc.declare_dram_parameter("ropeC", [128, S], F32, isOutput=False)
    ropeS = nc.declare_dram_parameter("ropeS", [128, S], F32, isOutput=False)
    perm = nc.declare_dram_parameter("perm", [128, 128], F16, isOutput=False)
    # oneh[:, 8r:8r+8] = one-hot col r (denominator row-select lhsT)
    oneh = nc.declare_dram_parameter("oneh", [128, 8 * (2 * NQT)], F16,
                                     isOutput=False)
    # sel[:, 128r:128(r+1)] = [8,128] with row r all-ones (bcast lhsT)
    sel = nc.declare_dram_parameter("sel", [8, 128 * (2 * NQT)], F32R,
                                    isOutput=False)
    if n_mask_tiles:
        maskt = nc.declare_dram_parameter(
            "maskt", [n_mask_tiles, KTILE, TOK], F16, isOutput=False)
    outT = nc.declare_dram_parameter("outT", [CW, T], F32, isOutput=True)

    # attention for q-tile qt can start once projections cover its last
    # active k-tile (for causal masks that is token-tile qt itself)
    maxtt = {}
    for qt in range(NQT):
        kts = [kt for kt in range(NKT) if not skip[kt, qt]]
        assert kts, f"fully masked q tile {qt}"
        maxtt[qt] = max(max(kts) // (TOK // KTILE), qt)
    attn_order = []
    for tl in range(NTB):
        for qt in range(NQT):
            if maxtt[qt] == tl:
                for h in range(HPC):
                    attn_order.append((qt, h))
    first_qh = attn_order[0]
    last_qh = attn_order[-1]
    half_a = [p for p in attn_order if p[0] < NQT // 2]
    last_a = half_a[-1] if half_a else None

    inv_sqrt_hd = 1.0 / float(np.sqrt(HD))

    with tile.TileContext(nc) as tc:
        with tc.tile_pool(name="const", bufs=1) as const, \
             tc.tile_pool(name="dram", bufs=1, space="DRAM") as dram:
            # persistent SBUF constants; weights split into 4 DMAs so the
            # first projection matmuls start as soon as chunk 0 lands
            wq_sb = const.tile([128, NKC * CW], F16)
            wk_sb = const.tile([128, NKC * CW], F16)
            wv_sb = const.tile([128, NKC * CW], F16)
            wo_sb = const.tile([128, NKC * CW], F16)
            Wq = NKC * CW // 4
            # sync carries only wq + the x tiles so the first matmul
            # starts ~9us in; other consts ride scalar/gpsimd queues
            for sb, dr, eng in ((wq_sb, wq, nc.sync),
                                (wk_sb, wk, nc.scalar),
                                (wv_sb, wv, nc.scalar)):
                for j in range(4):
                    eng.dma_start(sb[:, j * Wq:(j + 1) * Wq],
                                  dr[:, j * Wq:(j + 1) * Wq])
            C_sb = const.tile([128, S], F32)
            S_sb = const.tile([128, S], F32)
            nc.gpsimd.dma_start(C_sb[:], ropeC[:])
            nc.gpsimd.dma_start(S_sb[:], ropeS[:])
            perm_sb = const.tile([128, 128], F16)
            oneh_sb = const.tile([128, 8 * (2 * NQT)], F16)
            sel_sb = const.tile([8, 128 * (2 * NQT)], F32R)
            nc.gpsimd.dma_start(perm_sb[:], perm[:])
            nc.gpsimd.dma_start(oneh_sb[:], oneh[:])
            nc.gpsimd.dma_start(sel_sb[:], sel[:])
            preload_mask = 0 < n_mask_tiles <= 8
            mask_sb = None
            if preload_mask:
                mask_sb = const.tile([128, n_mask_tiles * TOK], F16)
                for j in range(n_mask_tiles):
                    nc.scalar.dma_start(
                        mask_sb[:, j * TOK:(j + 1) * TOK], maskt[j])

            # internal DRAM: one AllGather buffer pair per batch so the
            # collectives overlap with later batches' compute
            ag_in = [[dram.tile([CW, S // 2], F16, name=f"agin{b}_{hf}")
                      for hf in range(2)] for b in range(B)]
            ag_out = [[dram.tile([CW * N_CORES, S // 2], F16,
                                 addr_space="Shared", name=f"agout{b}_{hf}")
                       for hf in range(2)] for b in range(B)]

            # -------- fused per-batch: QKV projection + rope + attention ----
            with tc.tile_pool(name="fx", bufs=2) as fx, \
                 tc.tile_pool(name="fbig", bufs=2) as fbig, \
                 tc.tile_pool(name="facc", bufs=1, space="PSUM") as facc, \
                 tc.tile_pool(name="fscr", bufs=2, space="PSUM") as fscr, \
                 tc.tile_pool(name="fo", bufs=1, space="PSUM") as fo, \
                 tc.tile_pool(name="fd", bufs=1, space="PSUM") as fd, \
                 tc.tile_pool(name="fsb", bufs=2) as fsb, \
                 tc.tile_pool(name="fac", bufs=2) as fac, \
                 tc.tile_pool(name="fno", bufs=2 * NQT) as fno, \
                 tc.tile_pool(name="fex", bufs=3) as fex, \
                 tc.tile_pool(name="fxm", bufs=2) as fxm, \
                 tc.tile_pool(name="p3a", bufs=20) as p3a:

                def emit_norm(b, hf, drc_b, o_raws_b):
                    # normalize + store one half (q tiles 2hf..2hf+1) of
                    # batch b, then AllGather that half
                    for (qt, h) in attn_order:
                        if qt // 2 != hf:
                            continue
                        r = 2 * qt + h
                        bc2 = fscr.tile([128, 2 * TOK], F32, tag="scr2",
                                        name="bc")
                        bc_ps = bc2[:, 0:TOK]
                        kp = 4 if hf == 0 else 8
                        nc.tensor.matmul(
                            bc_ps, sel_sb[0:kp, 128 * r:128 * (r + 1)],
                            drc_b[0:kp, :], start=True, stop=True)
                        bcs = fsb.tile([128, TOK], F16, tag="bcs",
                                       name="bcs")
                        nc.vector.tensor_copy(bcs[:], bc_ps)
                        o_sb = fsb.tile([128, TOK], F16, tag="osb",
                                        name="osb")
                        nc.vector.tensor_mul(o_sb[:], o_raws_b[(qt, h)][:],
                                             bcs[:])
                        nc.gpsimd.dma_start(
                            ag_in[b][hf][128 * h:128 * (h + 1),
                                         TOK * (qt - 2 * hf):
                                         TOK * (qt - 2 * hf + 1)],
                            o_sb[:])
                    nc.gpsimd.collective_compute(
                        "AllGather", mybir.AluOpType.bypass,
                        ins=[ag_in[b][hf].opt()],
                        outs=[ag_out[b][hf].opt()],
                        replica_groups=[list(range(N_CORES))],
                    )

                def emit_p3(tt):
                    # one output-projection token tile; single PSUM bank
                    # (tag "ops"), m-halves sequential over resident ach
                    pb, ptl = tt // NTB, tt % NTB
                    hf, po = ptl // 2, ptl % 2
                    achs = []
                    for kc in range(NKC):
                        ach = p3a.tile([128, TOK], F16, tag="ach",
                                       name="ach")
                        nc_dma = nc.scalar if kc % 2 else nc.sync
                        nc_dma.dma_start(
                            ach[:],
                            ag_out[pb][hf][128 * kc:128 * (kc + 1),
                                           TOK * po:TOK * (po + 1)])
                        achs.append(ach)
                    for m in range(HPC):
                        pss = fo.tile([128, TOK], F32, tag="ops",
                                      name="pss")
                        for kc in range(NKC):
                            c0 = kc * CW + m * 128
                            nc.tensor.matmul(pss[:],
                                             wo_sb[:, c0:c0 + 128],
                                             achs[kc][:],
                                             start=(kc == 0),
                                             stop=(kc == NKC - 1))
                        osb = fsb.tile([128, TOK], F32, tag="osb3",
                                       name="osb3")
                        nc.scalar.copy(osb[:], pss[:])
                        nc.gpsimd.dma_start(
                            outT[128 * m:128 * (m + 1),
                                 TOK * tt:TOK * (tt + 1)], osb[:])

                pending_norm = None     # (b, drc, o_raws) awaiting emission
                pending_p3 = []         # token tiles whose AG has triggered
                # how many p3 tiles to fold into each tl slot (skip tl=0,
                # where the deferred normalize runs)
                p3_quota = {0: 0, 1: 0, 2: 2, 3: 2}
                for b in range(B):
                    qT = [fbig.tile([128, S], F16, tag=f"qT{h}", name=f"qT{h}")
                          for h in range(HPC)]
                    kT = [fbig.tile([128, S], F16, tag=f"kT{h}", name=f"kT{h}")
                          for h in range(HPC)]
                    vnat = [fbig.tile([128, S], F16, tag=f"vn{h}",
                                      name=f"vn{h}")
                            for h in range(HPC)]
                    d_all = fd.tile([2 * NQT, TOK], F32, tag="dps",
                                    name="dall")
                    drc = fsb.tile([2 * NQT, TOK], F32R, tag="drc",
                                   name="drc")
                    o_raws = {}
                    for tl in range(NTB):
                        tt = NTB * b + tl
                        pos0 = tl * TOK
                        xbig = fx.tile([128, XW], F16, tag="xbig",
                                       name="xbig")
                        nc.sync.dma_start(
                            xbig[:], xT[:, XW * tt:XW * (tt + 1)])

                        def xsl(kc):
                            return xbig[:, TOK * kc:TOK * (kc + 1)]

                        # Q pass then K pass, 2 shared psum banks each;
                        # rope follows each pass
                        for nm, wsb, dsts in (("q", wq_sb, qT),
                                              ("k", wk_sb, kT)):
                            pp = [facc.tile([128, TOK], F32, tag=f"pa{hh}",
                                            name=f"p{nm}{hh}")
                                  for hh in range(HPC)]
                            for kc in range(NKC):
                                st = (kc == 0)
                                sp = (kc == NKC - 1)
                                for hh in range(HPC):
                                    c0 = kc * CW + hh * HD
                                    nc.tensor.matmul(pp[hh][:],
                                                     wsb[:, c0:c0 + HD],
                                                     xsl(kc),
                                                     start=st, stop=sp)
                            for hh in range(HPC):
                                raw = fsb.tile([128, TOK], F16, tag="raw",
                                               name="raw")
                                nc.vector.tensor_copy(raw[:], pp[hh][:])
                                sw2 = fscr.tile([128, 2 * TOK], F32,
                                                tag="scr2", name="swp")
                                swp = sw2[:, 0:TOK]
                                nc.tensor.matmul(swp, perm_sb[:], raw[:],
                                                 start=True, stop=True)
                                t1 = fsb.tile([128, TOK], F16, tag="t1",
                                              name="t1")
                                nc.vector.tensor_mul(
                                    t1[:], pp[hh][:], C_sb[:, pos0:pos0 + TOK])
                                t2 = fsb.tile([128, TOK], F16, tag="t2",
                                              name="t2")
                                nc.vector.tensor_mul(
                                    t2[:], swp, S_sb[:, pos0:pos0 + TOK])
                                nc.vector.tensor_add(
                                    dsts[hh][:, pos0:pos0 + TOK],
                                    t1[:], t2[:])
                        if tl == 0 and pending_norm is not None:
                            pb_, drc_, oraws_ = pending_norm
                            emit_norm(pb_, 0, drc_, oraws_)
                            emit_norm(pb_, 1, drc_, oraws_)
                            pending_norm = None
                            pending_p3.extend(
                                NTB * (b - 1) + i for i in range(NTB))
                        if tl == 1 and b == 0:
                            for j in range(4):
                                nc.scalar.dma_start(
                                    wo_sb[:, j * Wq:(j + 1) * Wq],
                                    wo[:, j * Wq:(j + 1) * Wq])
                        if tl == 2 and b == B - 1:
                            # last batch: first half eagerly, so the final
                            # AllGather overlaps its own p3 tiles
                            emit_norm(b, 0, drc, o_raws)
                        # V pass: project directly into [token, feat] layout
                        # (stationary = x sub-block, moving = wv chunk row)
                        for sub in range(TOK // KTILE):
                            kt = (pos0 // KTILE) + sub
                            pv = facc.tile([128, CW], F32, tag="pa0",
                                           name="pav")
                            for kc in range(NKC):
                                nc.tensor.matmul(
                                    pv[:],
                                    xsl(kc)[:, KTILE * sub:KTILE * (sub + 1)],
                                    wv_sb[:, kc * CW:(kc + 1) * CW],
                                    start=(kc == 0), stop=(kc == NKC - 1))
                            for hh in range(HPC):
                                nc.vector.tensor_copy(
                                    vnat[hh][:, KTILE * kt:KTILE * (kt + 1)],
                                    pv[:, HD * hh:HD * (hh + 1)])
                        # attention for every q-tile whose K/V coverage is
                        # now complete; kt tiles processed in pairs with a
                        # single exp over [128, 2*TOK]
                        for qt in range(NQT):
                          if maxtt[qt] != tl:
                            continue
                          for h in range(HPC):
                            r = 2 * qt + h
                            kts = [kt for kt in range(NKT) if not skip[kt, qt]]
                            groups = [kts[i:i + 2]
                                      for i in range(0, len(kts), 2)]
                            o_ps = fo.tile([128, TOK], F32, tag="ops",
                                           name="ops")
                            acc = None
                            nmm = len(kts)
                            j = 0
                            for grp in groups:
                                ng = len(grp)
                                s2 = fscr.tile([128, 2 * TOK], F32,
                                               tag="scr2", name="sps")
                                for gi, kt in enumerate(grp):
                                    nc.tensor.matmul(
                                        s2[:, TOK * gi:TOK * (gi + 1)],
                                        kT[h][:, KTILE * kt:KTILE * (kt + 1)],
                                        qT[h][:, TOK * qt:TOK * (qt + 1)],
                                        start=True, stop=True)
                                e2 = fex.tile([128, 2 * TOK], F16, tag="ex",
                                              name="ex")
                                nc.scalar.activation(
                                    e2[:, 0:TOK * ng], s2[:, 0:TOK * ng],
                                    mybir.ActivationFunctionType.Exp,
                                    scale=inv_sqrt_hd)
                                for gi, kt in enumerate(grp):
                                    ex = e2[:, TOK * gi:TOK * (gi + 1)]
                                    if need[kt, qt]:
                                        mj = mask_uid[(kt, qt)]
                                        if preload_mask:
                                            msrc = mask_sb[:, mj * TOK:
                                                           (mj + 1) * TOK]
                                        else:
                                            mld = fsb.tile(
                                                [128, TOK], F16,
                                                tag="mld", name="mld")
                                            nc.sync.dma_start(mld[:],
                                                              maskt[mj])
                                            msrc = mld[:]
                                        exm = fxm.tile([128, TOK], F16,
                                                       tag="exm", name="exm")
                                        nc.vector.tensor_mul(exm[:], ex,
                                                             msrc)
                                        ex = exm[:]
                                    nc.tensor.matmul(
                                        o_ps[:],
                                        vnat[h][:, KTILE * kt:
                                                KTILE * (kt + 1)],
                                        ex, start=(j == 0),
                                        stop=(j == nmm - 1))
                                    # f16 running sum for the denominator
                                    nacc = fac.tile([128, TOK], F16,
                                                    tag="acb", name="acb")
                                    if acc is None:
                                        nc.vector.tensor_copy(nacc[:], ex)
                                    else:
                                        nc.vector.tensor_add(nacc[:],
                                                             acc[:], ex)
                                    acc = nacc
                                    j += 1
                            # one column-sum matmul per (qt,h) into row r
                            nc.tensor.matmul(
                                d_all[:], oneh_sb[:, 8 * r:8 * r + 8],
                                acc[:],
                                start=((qt, h) == first_qh),
                                stop=((qt, h) == last_qh))
                            # unnormalized AV -> SBUF, freeing the PSUM bank
                            orw = fno.tile([128, TOK], F16, tag="oraw",
                                           name="oraw")
                            nc.vector.tensor_copy(orw[:], o_ps[:])
                            o_raws[(qt, h)] = orw
                            # full-tile recips (partition slices must
                            # start at 0): at last_a rows 4-7 are still
                            # exactly 0 -> Inf, never read (half-a bcast
                            # contracts over partitions 0-3 only); the
                            # second recip refreshes rows 4-7
                            if (qt, h) == last_a or (qt, h) == last_qh:
                                with nc.allow_low_precision(
                                        reason="fp32r softmax denom recip"):
                                    nc.vector.reciprocal(drc[:], d_all[:])
                        # fold deferred output-projection tiles into
                        # this tl slot (PE-dense filler work)
                        for _ in range(p3_quota[tl]):
                            if pending_p3:
                                emit_p3(pending_p3.pop(0))
                    pending_norm = (b, drc, o_raws)
                # drain: last batch's second half, then its p3 tiles
                # (first two were AllGathered eagerly at tl==2)
                pb_, drc_, oraws_ = pending_norm
                emit_norm(pb_, 1, drc_, oraws_)
                pending_norm = None
                pending_p3.extend(NTB * (B - 1) + i for i in range(NTB))
                while pending_p3:
                    emit_p3(pending_p3.pop(0))

    nc.compile()
    return nc


def prepare(x, wq, wk, wv, wo, freqs_cos, freqs_sin, mask, cache_k, cache_v,
            start_pos):
    """Compile (cached) and build per-core input maps."""
    assert int(start_pos) == 0, "kernel compiled for start_pos == 0"
    x = np.asarray(x, dtype=np.float32)
    wq = np.asarray(wq, dtype=np.float32)
    wk = np.asarray(wk, dtype=np.float32)
    wv = np.asarray(wv, dtype=np.float32)
    wo = np.asarray(wo, dtype=np.float32)
    fc = np.asarray(freqs_cos, dtype=np.float32)
    fs = np.asarray(freqs_sin, dtype=np.float32)
    mask = np.asarray(mask, dtype=np.float32)

    skip, need = _analyze_mask(mask)
    with np.errstate(under="ignore", over="ignore"):
        mT = np.exp(mask.reshape(S, S).T.astype(np.float64))  # [k, q] mult
    uniq, mask_uid, tiles = {}, {}, []
    for kt in range(NKT):
        for qt in range(NQT):
            if need[kt, qt]:
                tl = np.ascontiguousarray(
                    mT[KTILE * kt:KTILE * (kt + 1),
                       TOK * qt:TOK * (qt + 1)]).astype(np.float16)
                hkey = tl.tobytes()
                if hkey not in uniq:
                    uniq[hkey] = len(tiles)
                    tiles.append(tl)
                mask_uid[(kt, qt)] = uniq[hkey]
    n_mask_tiles = len(tiles)

    key = (skip.tobytes(), need.tobytes(),
           tuple(sorted(mask_uid.items())))
    if key not in _KERNEL_CACHE:
        _KERNEL_CACHE[key] = _build(skip, need, n_mask_tiles, mask_uid)
    nc = _KERNEL_CACHE[key]

    # host-side input marshalling: x -> [128, tt, kc, c] interleaved
    x16 = x.reshape(T, D).astype(np.float16)
    xT = np.ascontiguousarray(
        x16.reshape(NTT, TOK, NKC, 128).transpose(3, 0, 2, 1).reshape(
            128, NTT * XW))
    C = np.repeat(fc.T, 2, axis=0).astype(np.float32)                # [128, S]
    Ssg = np.repeat(fs.T, 2, axis=0).astype(np.float32)
    Ssg[0::2] *= -1.0
    pm = np.zeros((128, 128), np.float16)
    idx = np.arange(0, 128, 2)
    pm[idx, idx + 1] = 1.0
    pm[idx + 1, idx] = 1.0
    oneh = np.zeros((128, 8 * (2 * NQT)), np.float16)
    sel = np.zeros((8, 128 * (2 * NQT)), np.float32)
    for r in range(2 * NQT):
        oneh[:, 8 * r + r] = 1.0
        sel[r, 128 * r:128 * (r + 1)] = 1.0
    maskt = (np.ascontiguousarray(np.stack(tiles)) if tiles
             else np.zeros((0, KTILE, TOK), np.float16))

    def chunk_w(w):  # [D, CW] -> [128, NKC*CW], fp16
        return np.ascontiguousarray(
            w.reshape(NKC, 128, CW).transpose(1, 0, 2).reshape(
                128, NKC * CW).astype(np.float16))

    in_maps = []
    for i in range(N_CORES):
        cols = slice(CW * i, CW * (i + 1))
        m = {
            "xT": xT,
            "wq": chunk_w(wq[:, cols]),
            "wk": chunk_w(wk[:, cols]),
            "wv": chunk_w(wv[:, cols]),
            "wo": chunk_w(wo[:, cols]),
            "ropeC": C, "ropeS": Ssg,
            "perm": pm, "oneh": oneh, "sel": sel,
        }
        if n_mask_tiles:
            m["maskt"] = maskt
        in_maps.append(m)
    return nc, in_maps


def kernel(**inputs):
    nc, in_maps = prepare(**inputs)
    res = run_bass_kernel_spmd(nc, in_maps, list(range(N_CORES)))
    outT = np.concatenate([res.results[i]["outT"] for i in range(N_CORES)],
                          axis=0)  # [D, T]
    return np.ascontiguousarray(outT.T).reshape(B, S, D)
